# revision 4
# baseline (speedup 1.0000x reference)
"""Bigram self-attention LM forward on 8 Trainium2 NeuronCores.

Sharding: core = 2*batch + vocab_half.  Each core computes its batch's
attention (duplicated across the 2 vocab halves) and its half of the
lm_head/softmax-stats.  Host assembles logits and combines the two
per-half sumexp vectors into the cross-entropy loss.

All matmuls run in float32r (full-rate fp32 streaming mode, ~1e-4 rel
err); attention probabilities and V are bf16 (error ~5e-4 on the
attention output, negligible after the lm_head).
"""
import numpy as np
import ml_dtypes

import jax
from jax.sharding import Mesh, PartitionSpec
from jax.experimental.shard_map import shard_map

import concourse.bacc as bacc
import concourse.tile as tile
import concourse.mybir as mybir
from concourse import bass2jax
from concourse.bass2jax import _bass_exec_p, partition_id_tensor, install_neuronx_cc_hook
from concourse.masks import make_identity

F32 = mybir.dt.float32
F32R = mybir.dt.float32r
BF16 = mybir.dt.bfloat16
I16 = mybir.dt.int16
EXP = mybir.ActivationFunctionType.Exp
COPY = mybir.ActivationFunctionType.Copy

B, T, C, V = 4, 2048, 1024, 32000
NCORES = 8
VH = V // 2           # 16000 per core
TC = T // 128         # 16 token tiles
CCH = C // 128        # 8 channel chunks
NTB = 4               # t-blocks of 512
SCALE = float(C) ** -0.5
NVT = (VH + 511) // 512  # 32 vocab chunks (31x512 + 1x128)


def _r(ap):
    return ap.bitcast(F32R)


def build_program(with_bias):
    nc = bacc.Bacc(None, target_bir_lowering=False)

    idx16 = nc.dram_tensor("idx16", [128, T // 16], I16, kind="ExternalInput")
    tok = nc.dram_tensor("tok_emb", [V, C], F32, kind="ExternalInput")
    pos = nc.dram_tensor("pos_emb", [T, C], F32, kind="ExternalInput")
    wq = nc.dram_tensor("wq", [C, C], F32R, kind="ExternalInput")
    wk = nc.dram_tensor("wk", [C, C], F32R, kind="ExternalInput")
    wv = nc.dram_tensor("wv", [C, C], F32R, kind="ExternalInput")
    wlm = nc.dram_tensor("wlm", [C, VH], F32R, kind="ExternalInput")
    maskb = nc.dram_tensor("maskb", [128, 896], BF16, kind="ExternalInput")
    if with_bias:
        blm = nc.dram_tensor("blm", [VH], F32, kind="ExternalInput")

    logits = nc.dram_tensor("logits", [T, VH], F32, kind="ExternalOutput")
    sumexp = nc.dram_tensor("sumexp", [128, TC], F32, kind="ExternalOutput")

    pos_r = pos.ap().rearrange("(tc p) d -> p tc d", p=128)
    wq_r = wq.ap().rearrange("(cc p) m -> p cc m", p=128)
    wk_r = wk.ap().rearrange("(cc p) m -> p cc m", p=128)
    wv_r = wv.ap().rearrange("(cc p) m -> p cc m", p=128)
    wlm_r = wlm.ap().rearrange("(cc p) v -> p cc v", p=128)
    log_r = logits.ap().rearrange("(tc p) v -> p tc v", p=128)

    with tile.TileContext(nc) as tc_:
        with (
            tc_.tile_pool(name="pmain", bufs=1) as pmain,
            tc_.tile_pool(name="pdram", bufs=1, space="DRAM") as pdram,
            tc_.tile_pool(name="pshare", bufs=1) as pshare,
        ):
            ident = pmain.tile([128, 128], F32)
            make_identity(nc, ident[:])
            maskt = pmain.tile([128, 896], BF16)
            nc.sync.dma_start(maskt[:], maskb.ap())
            ones_bf = pmain.tile([128, 1], BF16)
            nc.gpsimd.memset(ones_bf[:], 1.0)
            idxt = pmain.tile([128, T // 16], I16)
            nc.sync.dma_start(idxt[:], idx16.ap())
            recipz = pmain.tile([128, TC], F32)
            sume = pmain.tile([128, TC, NVT], F32)

            qt_dram = pdram.tile([NTB, CCH, 128, 512], F32R)
            v_dram = pdram.tile([TC, 128, C], BF16)

            xT = pshare.tile([128, CCH, T], F32R, tag="big")

            # ---- Phase A+B: gather + pos add + transpose into xT ----
            with (
                tc_.tile_pool(name="px", bufs=1) as px,
                tc_.tile_pool(name="ppos", bufs=2) as ppos,
                tc_.tile_pool(name="ppsT", bufs=8, space="PSUM") as ppsT,
            ):
                for h in range(2):
                    x_sb = px.tile([128, 8, C], F32, tag="x")
                    nc.gpsimd.dma_gather(
                        x_sb[:], tok.ap(), idxt[:, 64 * h : 64 * h + 64],
                        1024, 1024, C,
                    )
                    for c in range(8):
                        tc = 8 * h + c
                        pt = ppos.tile([128, C], F32, tag="pos")
                        nc.sync.dma_start(pt[:], pos_r[:, tc, :])
                        nc.vector.tensor_add(x_sb[:, c, :], x_sb[:, c, :], pt[:])
                        for cc in range(CCH):
                            ps = ppsT.tile([128, 128], F32, tag="ps")
                            nc.tensor.transpose(
                                ps[:], x_sb[:, c, cc * 128 : cc * 128 + 128], ident[:]
                            )
                            nc.scalar.copy(
                                xT[:, cc, tc * 128 : tc * 128 + 128], ps[:]
                            )

            with tc_.tile_pool(name="pkT", bufs=1) as pkT:
                kT = pkT.tile([128, CCH, T], F32R)

                # ---- Phase C: kT, v, qT ----
                with (
                    tc_.tile_pool(name="pw", bufs=2) as pw,
                    tc_.tile_pool(name="pst", bufs=3) as pst,
                    tc_.tile_pool(name="ppsC", bufs=3, space="PSUM") as ppsC,
                ):
                    # kT[co, t] = sum_ci Wk[ci, co] * xT[ci, t]
                    for wc in range(2):
                        wt = pw.tile([128, CCH, 512], F32R, tag="w")
                        nc.sync.dma_start(wt[:], wk_r[:, :, wc * 512 : wc * 512 + 512])
                        for col in range(4):
                            co = wc * 4 + col
                            for tb in range(NTB):
                                ps = ppsC.tile([128, 512], F32, tag="psc")
                                for ci in range(CCH):
                                    nc.tensor.matmul(
                                        ps[:],
                                        wt[:, ci, col * 128 : col * 128 + 128],
                                        xT[:, ci, tb * 512 : tb * 512 + 512],
                                        start=(ci == 0), stop=(ci == CCH - 1),
                                    )
                                nc.scalar.copy(kT[:, co, tb * 512 : tb * 512 + 512], ps[:])
                    # v[t, c] (bf16, to DRAM)
                    for wc in range(2):
                        wt = pw.tile([128, CCH, 512], F32R, tag="w")
                        nc.sync.dma_start(wt[:], wv_r[:, :, wc * 512 : wc * 512 + 512])
                        for tc in range(TC):
                            vst = pst.tile([128, 512], BF16, tag="vst")
                            ps = ppsC.tile([128, 512], F32, tag="psc")
                            for ci in range(CCH):
                                nc.tensor.matmul(
                                    ps[:],
                                    xT[:, ci, tc * 128 : tc * 128 + 128],
                                    wt[:, ci, :],
                                    start=(ci == 0), stop=(ci == CCH - 1),
                                )
                            nc.scalar.copy(vst[:], ps[:])
                            nc.sync.dma_start(
                                v_dram[tc, :, wc * 512 : wc * 512 + 512], vst[:]
                            )
                    # qT -> DRAM blocks
                    for wc in range(2):
                        wt = pw.tile([128, CCH, 512], F32R, tag="w")
                        nc.sync.dma_start(wt[:], wq_r[:, :, wc * 512 : wc * 512 + 512])
                        for col in range(4):
                            co = wc * 4 + col
                            for tb in range(NTB):
                                ps = ppsC.tile([128, 512], F32, tag="psc")
                                for ci in range(CCH):
                                    nc.tensor.matmul(
                                        ps[:],
                                        wt[:, ci, col * 128 : col * 128 + 128],
                                        xT[:, ci, tb * 512 : tb * 512 + 512],
                                        start=(ci == 0), stop=(ci == CCH - 1),
                                    )
                                qst = pst.tile([128, 512], F32R, tag="qst")
                                nc.scalar.copy(qst[:], ps[:])
                                nc.sync.dma_start(qt_dram[tb, co], qst[:])

                # ---- Phase D+E: attention per t-block ----
                outT = pshare.tile([128, CCH, T], F32R, tag="big")
                with (
                    tc_.tile_pool(name="pexpw", bufs=2) as pexpw,
                    tc_.tile_pool(name="pqb", bufs=1) as pqb,
                    tc_.tile_pool(name="pv", bufs=3) as pv,
                    tc_.tile_pool(name="ppsS", bufs=2, space="PSUM") as ppsS,
                    tc_.tile_pool(name="ppsZ", bufs=1, space="PSUM") as ppsZ,
                    tc_.tile_pool(name="ppsA", bufs=1, space="PSUM") as ppsA,
                ):
                    for j in range(NTB):
                        nsi = 4 * j + 4
                        qb = pqb.tile([128, CCH, 512], F32R, tag="qb")
                        nc.sync.dma_start(
                            qb[:], qt_dram[j].rearrange("cc p t -> p cc t")
                        )
                        expw = pexpw.tile([128, TC, 512], BF16, tag="expw")
                        for si in range(nsi):
                            ps = ppsS.tile([128, 512], F32, tag="pss")
                            for ci in range(CCH):
                                nc.tensor.matmul(
                                    ps[:],
                                    kT[:, ci, si * 128 : si * 128 + 128],
                                    qb[:, ci, :],
                                    start=(ci == 0), stop=(ci == CCH - 1),
                                )
                            nc.scalar.activation(expw[:, si, :], ps[:], EXP, scale=SCALE)
                            if si >= 4 * j:
                                off = 384 - (si * 128 - j * 512)
                                nc.vector.tensor_mul(
                                    expw[:, si, :], expw[:, si, :],
                                    maskt[:, off : off + 512],
                                )
                        for tl in range(4):
                            tt = 4 * j + tl
                            pz = ppsZ.tile([128, 1], F32, tag="pz")
                            for si in range(nsi):
                                nc.tensor.matmul(
                                    pz[:],
                                    expw[:, si, tl * 128 : tl * 128 + 128],
                                    ones_bf[:, 0:1],
                                    start=(si == 0), stop=(si == nsi - 1),
                                )
                            nc.vector.reciprocal(recipz[:, tt : tt + 1], pz[:])
                        for half in range(2):
                            pavs = []
                            for i in range(4):
                                av_t = ppsA.tile([128, 512], F32, tag=f"av{i}", name=f"av{i}_{j}_{half}")
                                pavs.append(av_t)
                            for si in range(nsi):
                                vt_ = pv.tile([128, C], BF16, tag="vt")
                                nc.sync.dma_start(vt_[:], v_dram[si])
                                for c4 in range(4):
                                    cc = half * 4 + c4
                                    nc.tensor.matmul(
                                        pavs[c4][:],
                                        vt_[:, cc * 128 : cc * 128 + 128],
                                        expw[:, si, :],
                                        start=(si == 0), stop=(si == nsi - 1),
                                    )
                            for c4 in range(4):
                                cc = half * 4 + c4
                                nc.scalar.copy(
                                    outT[:, cc, j * 512 : j * 512 + 512], pavs[c4][:]
                                )

            # ---- Phase F: lm_head + loss stats ----
            with (
                tc_.tile_pool(name="pwlm", bufs=3) as pwlm,
                tc_.tile_pool(name="plog", bufs=2) as plog,
                tc_.tile_pool(name="pjunk", bufs=1) as pjunk,
                tc_.tile_pool(name="pblm", bufs=2) as pblm,
                tc_.tile_pool(name="ppsF", bufs=4, space="PSUM") as ppsF,
            ):
                junk = pjunk.tile([128, 512], BF16)
                for vt in range(NVT):
                    v0 = vt * 512
                    nvt = min(512, VH - v0)
                    wt = pwlm.tile([128, CCH, 512], F32R, tag="wlm")
                    nc.sync.dma_start(wt[:, :, :nvt], wlm_r[:, :, v0 : v0 + nvt])
                    if with_bias:
                        bt = pblm.tile([128, 512], F32, tag="blm")
                        nc.sync.dma_start(
                            bt[:1, :nvt], blm.ap()[v0 : v0 + nvt][None, :]
                        )
                    lg = plog.tile([128, TC, 512], F32, tag="lg")
                    for tt in range(TC):
                        ps = ppsF.tile([128, 512], F32, tag="psf")
                        for ci in range(CCH):
                            nc.tensor.matmul(
                                ps[:, :nvt],
                                outT[:, ci, tt * 128 : tt * 128 + 128],
                                wt[:, ci, :nvt],
                                start=(ci == 0), stop=(ci == CCH - 1),
                            )
                        nc.vector.tensor_scalar_mul(
                            lg[:, tt, :nvt], ps[:, :nvt], recipz[:, tt : tt + 1]
                        )
                        if with_bias:
                            nc.vector.tensor_add(
                                lg[:, tt, :nvt], lg[:, tt, :nvt],
                                bt[:1, :nvt].partition_broadcast(128),
                            )
                            nc.scalar.activation(
                                junk[:, :nvt], lg[:, tt, :nvt], EXP,
                                accum_out=sume[:, tt, vt : vt + 1],
                            )
                        else:
                            nc.scalar.activation(
                                junk[:, :nvt], ps[:, :nvt], EXP,
                                scale=recipz[:, tt : tt + 1],
                                accum_out=sume[:, tt, vt : vt + 1],
                            )
                    nc.sync.dma_start(log_r[:, :, v0 : v0 + nvt], lg[:, :, :nvt])
                # reduce sumexp over vocab chunks
                se = pjunk.tile([128, TC], F32)
                nc.vector.tensor_reduce(
                    se[:], sume[:], axis=mybir.AxisListType.X, op=mybir.AluOpType.add
                )
                nc.sync.dma_start(sumexp.ap(), se[:])
    nc.compile()
    return nc


class _Runner:
    def __init__(self, nc, n_cores):
        install_neuronx_cc_hook()
        self.nc = nc
        self.n_cores = n_cores
        partition_name = nc.partition_id_tensor.name if nc.partition_id_tensor else None
        in_names, out_names, out_avals, zero_outs = [], [], [], []
        for alloc in nc.m.functions[0].allocations:
            if not isinstance(alloc, mybir.MemoryLocationSet):
                continue
            name = alloc.memorylocations[0].name
            if alloc.kind == "ExternalInput":
                if name != partition_name:
                    in_names.append(name)
            elif alloc.kind == "ExternalOutput":
                out_names.append(name)
                shape = tuple(alloc.tensor_shape)
                dtype = mybir.dt.np(alloc.dtype)
                out_avals.append(jax.core.ShapedArray(shape, dtype))
                zero_outs.append(np.zeros(shape, dtype))
        self.in_names, self.out_names = in_names, out_names
        self.out_avals, self.zero_outs = out_avals, zero_outs
        n_params, n_outs = len(in_names), len(out_avals)
        self.n_params = n_params
        donate = tuple(range(n_params, n_params + n_outs))
        bind_in_names = list(in_names) + list(out_names)
        if partition_name is not None:
            bind_in_names.append(partition_name)

        def _body(*args):
            operands = list(args)
            if partition_name is not None:
                operands.append(partition_id_tensor())
            outs = _bass_exec_p.bind(
                *operands,
                out_avals=tuple(out_avals),
                in_names=tuple(bind_in_names),
                out_names=tuple(out_names),
                lowering_input_output_aliases=(),
                sim_require_finite=True,
                sim_require_nnan=True,
                nc=nc,
            )
            return tuple(outs)

        devices = jax.devices()[:n_cores]
        self.devices = devices
        self.mesh = Mesh(np.asarray(devices), ("core",))
        in_specs = (PartitionSpec("core"),) * (n_params + n_outs)
        out_specs = (PartitionSpec("core"),) * n_outs
        self.fn = jax.jit(
            shard_map(_body, mesh=self.mesh, in_specs=in_specs,
                      out_specs=out_specs, check_rep=False),
            donate_argnums=donate, keep_unused=True,
        )

    def stage_inputs(self, in_maps):
        per_core = [[np.asarray(m[name]) for name in self.in_names] for m in in_maps]
        arrs = [np.concatenate([per_core[c][i] for c in range(self.n_cores)], axis=0)
                for i in range(self.n_params)]
        sh = jax.sharding.NamedSharding(self.mesh, PartitionSpec("core"))
        staged = [jax.device_put(a, sh) for a in arrs]
        jax.block_until_ready(staged)
        return staged

    def make_zeros(self):
        sh = jax.sharding.NamedSharding(self.mesh, PartitionSpec("core"))
        zs = [jax.device_put(np.zeros((self.n_cores * z.shape[0], *z.shape[1:]), z.dtype), sh)
              for z in self.zero_outs]
        jax.block_until_ready(zs)
        return zs

    def run(self, staged, zeros):
        outs = self.fn(*staged, *zeros)
        jax.block_until_ready(outs)
        return outs

    def results(self, outs):
        res = []
        for c in range(self.n_cores):
            d = {}
            for i, name in enumerate(self.out_names):
                a = np.asarray(outs[i])
                a = a.reshape(self.n_cores, *self.out_avals[i].shape)[c]
                d[name] = a
            res.append(d)
        return res


_CACHE = {}


def _get_runner(with_bias):
    key = bool(with_bias)
    if key not in _CACHE:
        _CACHE[key] = _Runner(build_program(key), NCORES)
    return _CACHE[key]


def _make_mask():
    # maskb[p, u] = 1.0 if (u - 384) >= (p - 384) ... slice [384-d : 896-d]
    # gives mask_d[p, ul] = (ul >= p + d)
    u = np.arange(896)[None, :]
    p = np.arange(128)[:, None]
    return (u >= p + 384).astype(ml_dtypes.bfloat16)


def make_in_maps(idx, tok_emb, pos_emb, Wk, Wq, Wv, Wlm, blm):
    with_bias = bool(np.any(blm))
    maskb = _make_mask()
    in_maps = []
    for core in range(NCORES):
        b, h = core // 2, core % 2
        ii = np.ascontiguousarray(idx[b].astype(np.int16))
        idx16 = np.tile(ii.reshape(T // 16, 16).T, (8, 1)).copy()
        m = {
            "idx16": idx16,
            "tok_emb": tok_emb,
            "pos_emb": pos_emb,
            "wq": Wq, "wk": Wk, "wv": Wv,
            "wlm": np.ascontiguousarray(Wlm[:, h * VH : (h + 1) * VH]),
            "maskb": maskb,
        }
        if with_bias:
            m["blm"] = np.ascontiguousarray(blm[h * VH : (h + 1) * VH])
        in_maps.append(m)
    return with_bias, in_maps


def kernel(idx, targets, tok_emb, pos_emb, Wk, Wq, Wv, Wlm, blm):
    idx = np.asarray(idx); targets = np.asarray(targets)
    tok_emb = np.asarray(tok_emb, dtype=np.float32)
    pos_emb = np.asarray(pos_emb, dtype=np.float32)
    Wk = np.asarray(Wk, dtype=np.float32); Wq = np.asarray(Wq, dtype=np.float32)
    Wv = np.asarray(Wv, dtype=np.float32); Wlm = np.asarray(Wlm, dtype=np.float32)
    blm = np.asarray(blm, dtype=np.float32)

    with_bias, in_maps = make_in_maps(idx, tok_emb, pos_emb, Wk, Wq, Wv, Wlm, blm)
    r = _get_runner(with_bias)
    staged = r.stage_inputs(in_maps)
    outs = r.run(staged, r.make_zeros())
    res = r.results(outs)

    logits2d = np.empty((B * T, V), np.float32)
    S = np.empty((B, T), np.float32)
    for core in range(NCORES):
        b, h = core // 2, core % 2
        logits2d[b * T : (b + 1) * T, h * VH : (h + 1) * VH] = res[core]["logits"]
        se = res[core]["sumexp"]  # [128, TC], t = tt*128 + p
        sev = se.T.reshape(T)
        if h == 0:
            S[b] = sev
        else:
            S[b] += sev
    logz = np.log(S).reshape(B * T)
    tgt = targets.reshape(B * T).astype(np.int64)
    tgt_logit = logits2d[np.arange(B * T), tgt]
    loss = np.float32(-(tgt_logit - logz).mean())
    return logits2d, loss
